# revision 2
# baseline (speedup 1.0000x reference)
"""CapsuleLayer (dynamic routing, 3 iterations) on 8 Trainium2 NeuronCores.

Sharding: hybrid 2 n-halves x 4 b-quarters.  Cores in a pair hold
complementary halves of the input capsules for the same batch quarter, so
the two routing AllReduces run over 2-rank groups (~2 ring steps, ~16us)
instead of an 8-rank ring (~140us).  Payload is s = [16, 1024] f32.

Pipeline per core (partitions = (n8, b16); free = (c, j) c-major, bf16):
  - Build: u_hat = x*W via block-diagonal-stationary matmuls (K = 8 n's x
    16 i); a dense x pack accumulates s0 = sum_n u_hat in the same pass.
    W streams as bf16 1024-col moving operands; PSUM evacuates to SBUF
    through ScalarE.
  - Routing sweeps are batched 3 groups per instruction to amortize the
    ~0.7us/instruction engine overhead: t1 = u*v (flat [128,3072] against
    a materialized v-replica), one tensor_reduce over j for the logits,
    ScalarE exp, 1/z, coefficients; t2 = u*c uses the fast inner-step-0
    broadcast form split between VectorE and the Pool engine; the n-sum
    runs on the PE as a delta-stationary matmul accumulating in PSUM.
  - The third routing iteration's reduce + squash happen on host: cores
    ship raw per-half partials s2 = [16, 1024].
"""

import numpy as np
from contextlib import ExitStack

import ml_dtypes

import concourse.bass as bass
import concourse.mybir as mybir
from concourse import tile
from concourse.bass_utils import run_bass_kernel_spmd
from concourse.vector_clock import ScopedClock

# Problem constants
B, N, Di = 64, 1152, 16
C, Dc = 32, 32
NCORES = 8
G2 = 2                       # n-halves
K4 = 4                       # b-quarters
NLOC = N // G2               # 576 input capsules per core
NG = NLOC // 8               # 72 groups of 8 n's
BLOC = B // K4               # 16 samples per core
EPS = 1e-7

F32 = mybir.dt.float32
BF16 = mybir.dt.bfloat16


class PatchedTC(tile.TileContext):
    """This walrus build only supports ONE sync-wait per instruction; Tile's
    final drain carries one wait per outstanding DMA-queue semaphore.  Split
    the extras onto single-wait SP nops."""

    def _drain_and_barrier(self, tick_clock, wait_clock):
        nc = self.nc
        drain_inst = nc.sync.drain()
        wait_clock.add_sem_waits(
            drain_inst.ins, ScopedClock({None: tick_clock.global_clock})
        )
        si = drain_inst.ins.sync_info
        if si is not None and len(si.on_wait) > 1:
            waits = list(si.on_wait)
            del si.on_wait[1:]
            for w in waits[1:]:
                n2 = nc.sync.nop()
                if n2.ins.sync_info is None:
                    n2.ins.sync_info = mybir.SyncInfo(on_update=[], on_wait=[w])
                else:
                    n2.ins.sync_info.on_wait.append(w)
        nc.all_engine_barrier()
        popped = nc._tile_sem_poison_stack.pop()
        assert popped is self._sem_poison
        nc.clear_and_free_semaphores(list(self.sems.allocated().values()))
        nc.all_engine_barrier()


def _split_multi_waits(nc):
    """Post-pass: any instruction carrying >1 sync wait gets the extras moved
    onto same-engine nop instructions inserted right before it."""
    for fn in nc.m.functions:
        for bb in fn.blocks:
            insts = list(bb.instructions)
            out = []
            for ins in insts:
                si = getattr(ins, "sync_info", None)
                if si is not None and si.on_wait is not None and len(si.on_wait) > 1:
                    waits = list(si.on_wait)
                    del si.on_wait[1:]
                    for k, w in enumerate(waits[1:]):
                        nop = mybir.InstNoOp(
                            name=f"{ins.name}-wsplit{k}", ins=[], outs=[]
                        )
                        nop.engine = ins.engine
                        nop.sync_info = mybir.SyncInfo(on_update=[], on_wait=[w])
                        out.append(nop)
                out.append(ins)
            if len(out) != len(insts):
                bb.instructions[:] = out


def _view_cj(ap, c=32, j=32):
    """[P, c*j] AP (c-major) -> [P, c, j]."""
    return ap.rearrange("p (c j) -> p c j", c=c, j=j)


def _rep_outer_top(ap, n):
    """[P, F] AP -> [P, n(step 0), F] broadcast view."""
    lst = [list(p) for p in ap.ap]
    new = [lst[0], [0, n]] + lst[1:]
    return bass.AP(ap.tensor, ap.offset, new)


def _bcast_inner(ap, n):
    """[P, C] AP -> [P, C, n(step 0)] broadcast view."""
    lst = [list(p) for p in ap.ap]
    return bass.AP(ap.tensor, ap.offset, lst + [[0, n]])


def build_program(repeat=1, no_ar=False, evac="act",
                  t1_pool_num=3, t1_pool_den=5,
                  n_sweeps=2, do_build=True, empty=False,
                  sweep_mode="batch", t2_dve_num=27,
                  dup_a=False, psum2=False,
                  softmax_mode="rd", red_pool_num=0, red_pool_den=5,
                  t2_pool_num=5, t2_pool_den=5, expand_mult=True,
                  batch_gb=3, batch_mx=False):
    nc = bass.Bass()

    w_pack = nc.declare_dram_parameter("w_pack", [NG, 128, 1024], BF16, isOutput=False)
    x_pack = nc.declare_dram_parameter("x_pack", [NG, 128, 144], BF16, isOutput=False)
    delta = nc.declare_dram_parameter("delta", [128, 16], BF16, isOutput=False)
    out_ext = nc.declare_dram_parameter("out", [BLOC, 1024], F32, isOutput=True)

    ctx = ExitStack()
    with PatchedTC(nc) as tc, ctx:
        sb = ctx.enter_context(tc.tile_pool(name="sb", bufs=1))
        wpool = ctx.enter_context(tc.tile_pool(name="w", bufs=3))
        xpool = ctx.enter_context(tc.tile_pool(name="x", bufs=3))
        psum_u = ctx.enter_context(
            tc.tile_pool(name="psu", bufs=2 if psum2 else 3, space="PSUM")
        )
        psum_s = ctx.enter_context(
            tc.tile_pool(name="pss", bufs=2 if psum2 else 1, space="PSUM")
        )
        tpool = ctx.enter_context(tc.tile_pool(name="t", bufs=4))
        btpool = ctx.enter_context(tc.tile_pool(name="bt", bufs=2))
        smpool = ctx.enter_context(tc.tile_pool(name="sm", bufs=4))
        epool = ctx.enter_context(tc.tile_pool(name="e", bufs=4))
        dram = ctx.enter_context(tc.tile_pool(name="dram", bufs=1, space="DRAM"))

        # Persistent SBUF (u_hat split: finer dependency granularity; part
        # size is a multiple of the batch width so blocks never straddle)
        n_uparts = 4 if batch_gb == 3 else 6
        u_parts = [
            sb.tile([128, (NG // n_uparts) * 1024], BF16, tag=f"uhat{i}",
                    name=f"u_sb{i}")
            for i in range(n_uparts)
        ]
        b1_sb = sb.tile([128, NG * 32], BF16, tag="b1")          # 4.5 KB/part
        delta_sb = sb.tile([128, 16], BF16, tag="delta")
        vb_sb = sb.tile([128, 1024], BF16, tag="vbcast")
        vb3_sb = sb.tile([128, 3072], BF16, tag="vb3")
        s_sb = sb.tile([BLOC, 1024], F32, tag="sfull")
        sq_sb = sb.tile([BLOC, 1024], BF16, tag="sq")
        n2_sb = sb.tile([BLOC, 96], F32, tag="n2")

        nc.sync.dma_start(out=delta_sb[:], in_=delta[:])
        if not do_build and not empty:
            for p in u_parts:
                nc.vector.memset(p[:], 0.01)

        def u_slice(g):
            psz = NG // n_uparts
            part, off = g // psz, g % psz
            return u_parts[part][:, off * 1024:(off + 1) * 1024]

        def b1_slice(g):
            return b1_sb[:, g * 32:(g + 1) * 32]

        # ---------- Phase 1: u_hat build + s0 accumulation ----------
        def build_uhat():
            ps_s0 = psum_s.tile([BLOC, 1024], F32, tag="s")
            for g in range(NG):
                w_t = wpool.tile([128, 1024], BF16, tag="w")
                nc.sync.dma_start(out=w_t[:], in_=w_pack[g])
                x_t = xpool.tile([128, 144], BF16, tag="x")
                nc.sync.dma_start(out=x_t[:], in_=x_pack[g])
                ps_u = psum_u.tile([128, 1024], F32, tag="u")
                for h in range(2):
                    sl = slice(h * 512, (h + 1) * 512)
                    nc.tensor.matmul(
                        ps_u[:, sl], x_t[:, 0:128], w_t[:, sl],
                        start=True, stop=True,
                    )
                    nc.tensor.matmul(
                        ps_s0[:, sl], x_t[:, 128:144], w_t[:, sl],
                        start=(g == 0), stop=(g == NG - 1),
                    )
                usl = u_slice(g)
                if evac == "act":
                    nc.scalar.copy(usl[:], ps_u[:])
                else:
                    nc.vector.tensor_copy(usl[:, 0:512], ps_u[:, 0:512])
                    nc.scalar.copy(usl[:, 512:1024], ps_u[:, 512:1024])
            return ps_s0

        # ---------- AllReduce + squash ----------
        def allreduce_squash(ps_s, scale0):
            bounce_in = dram.tile([BLOC, 1024], F32, tag="cin")
            bounce_out = dram.tile([BLOC, 1024], F32, tag="cout")
            nc.vector.tensor_scalar(
                s_sb[:], ps_s[:], scale0, None, mybir.AluOpType.mult
            )
            nc.sync.dma_start(out=bounce_in[:], in_=s_sb[:])
            if no_ar:
                nc.sync.dma_start(out=bounce_out[:], in_=bounce_in[:])
            else:
                nc.gpsimd.collective_compute(
                    "AllReduce",
                    mybir.AluOpType.add,
                    replica_groups=[[0, 1], [2, 3], [4, 5], [6, 7]],
                    ins=[bounce_in[:]],
                    outs=[bounce_out[:]],
                )
            nc.sync.dma_start(out=s_sb[:], in_=bounce_out[:])
            # squash: n2 = sum_j s^2 ; vbf = bf16(s * n2/(1+n2)/sqrt(n2+eps))
            with nc.allow_low_precision(reason="bf16 squares"):
                nc.vector.tensor_mul(sq_sb[:], s_sb[:], s_sb[:])
            n2 = n2_sb[:, 0:32]
            nc.vector.tensor_reduce(
                n2, _view_cj(sq_sb[:]), mybir.AxisListType.X,
                mybir.AluOpType.add,
            )
            rt = n2_sb[:, 32:64]
            nc.vector.tensor_scalar(rt, n2, EPS, None, mybir.AluOpType.add)
            nc.scalar.activation(rt, rt, mybir.ActivationFunctionType.Sqrt)
            on2 = n2_sb[:, 64:96]
            nc.vector.tensor_scalar(on2, n2, 1.0, None, mybir.AluOpType.add)
            nc.vector.tensor_mul(rt, rt, on2)
            nc.vector.reciprocal(rt, rt)
            nc.vector.tensor_mul(n2, n2, rt)   # n2 <- scale factor
            nc.vector.tensor_tensor(
                _view_cj(vb_sb[0:16, :]), _view_cj(s_sb[:]),
                _bcast_inner(n2, 32), mybir.AluOpType.mult,
            )
            # broadcast over the remaining 7 n8 partition blocks
            for n8 in range(1, 8):
                nc.sync.dma_start(
                    out=vb_sb[n8 * 16:(n8 + 1) * 16, :], in_=vb_sb[0:16, :],
                )
            nc.scalar.copy(
                vb3_sb[:].rearrange("p (g f) -> p g f", g=3),
                _rep_outer_top(vb_sb[:], 3),
            )

        # ---------- Sweep ----------
        # Per-g softmax is folded as: e = exp(logits) (bf16), z = sum_c e,
        # rd = delta * (1/z)  -> s += rd^T @ (u * e): the 1/z normalization
        # rides in the matmul stationary, shortening the cross-engine chain
        # (t2 depends only on e).

        def stage1(g, is_b):
            """t1 = u*v, logits reduce, softmax -> (mult_g, stat_g) tiles.

            softmax_mode 'rd': mult = e (bf16), stat = delta * (1/z).
            softmax_mode 'cmul': mult = c = e/z (via ACT), stat = delta.
            """
            t1 = tpool.tile([128, 1024], BF16, tag="t1")
            t1_eng = (
                nc.gpsimd if (g * t1_pool_num) % t1_pool_den < t1_pool_num
                else nc.vector
            )
            t1_eng.tensor_tensor(
                t1[:], u_slice(g)[:], vb_sb[:], mybir.AluOpType.mult,
            )
            red_eng = (
                nc.gpsimd if (g * red_pool_num) % red_pool_den < red_pool_num
                else nc.vector
            )
            if not is_b:
                lg = b1_slice(g)
                with nc.allow_low_precision(reason="bf16 logits store"):
                    red_eng.tensor_reduce(
                        lg, _view_cj(t1[:]), mybir.AxisListType.X,
                        mybir.AluOpType.add,
                    )
            else:
                b2 = smpool.tile([128, 32], F32, tag="b2")
                red_eng.tensor_reduce(
                    b2[:], _view_cj(t1[:]), mybir.AxisListType.X,
                    mybir.AluOpType.add,
                )
                lg = b2[:]
                nc.vector.tensor_add(lg, lg, b1_slice(g))
            e_g = epool.tile([128, 32], BF16, tag="e")
            z_g = smpool.tile([128, 1], F32, tag="z")
            nc.scalar.activation(
                e_g[:], lg, mybir.ActivationFunctionType.Exp,
                accum_out=z_g[:],
            )
            r_g = smpool.tile([128, 1], F32, tag="r")
            nc.vector.reciprocal(r_g[:], z_g[:])
            if softmax_mode == "rd":
                rd_g = epool.tile([128, 16], BF16, tag="rd")
                nc.vector.tensor_scalar(
                    rd_g[:], delta_sb[:], r_g[:], None, mybir.AluOpType.mult
                )
                return e_g, rd_g
            else:
                c_g = epool.tile([128, 32], BF16, tag="c")
                nc.scalar.mul(c_g[:], e_g[:], r_g[:])
                return c_g, None

        def stage2(g, ps_s, t2_eng, mult_g, stat_g):
            """t2 = u*mult, (weighted-)delta matmul accumulate."""
            t2 = tpool.tile([128, 1024], BF16, tag="t2")
            if expand_mult:
                mx = tpool.tile([128, 1024], BF16, tag="mx")
                nc.scalar.copy(
                    _view_cj(mx[:]), _bcast_inner(mult_g[:], 32),
                )
                t2_eng.tensor_tensor(
                    t2[:], u_slice(g)[:], mx[:], mybir.AluOpType.mult,
                )
            else:
                t2_eng.tensor_tensor(
                    _view_cj(t2[:]), _view_cj(u_slice(g)[:]),
                    _bcast_inner(mult_g[:], 32), mybir.AluOpType.mult,
                )
            stat = stat_g[:] if stat_g is not None else delta_sb[:]
            for h in range(2):
                sl = slice(h * 512, (h + 1) * 512)
                nc.tensor.matmul(
                    ps_s[:, sl], stat, t2[:, sl],
                    start=(g == 0), stop=(g == NG - 1),
                )

        def t2_engine(g):
            return (
                nc.gpsimd if (g * t2_pool_num) % t2_pool_den < t2_pool_num
                else nc.vector
            )

        # ---------- Batched sweep: GB groups per op ----------
        GB = batch_gb
        NB = NG // GB

        def u3_slice(k):
            g0 = k * GB
            psz = NG // n_uparts
            part, off = g0 // psz, g0 % psz
            return u_parts[part][:, off * 1024:(off + GB) * 1024]

        def _rep_outer(ap, n):
            """[P, F] AP -> [P, n(step 0), F] broadcast view."""
            lst = [list(p) for p in ap.ap]
            new = [lst[0], [0, n]] + lst[1:]
            return bass.AP(ap.tensor, ap.offset, new)

        def bstage1(k, is_b):
            t1 = btpool.tile([128, GB * 1024], BF16, tag="bt1")
            eng = (
                nc.gpsimd if (k * t1_pool_num) % t1_pool_den < t1_pool_num
                else nc.vector
            )
            if GB == 3:
                eng.tensor_tensor(
                    t1[:], u3_slice(k), vb3_sb[:], mybir.AluOpType.mult,
                )
            else:
                eng.tensor_tensor(
                    t1[:].rearrange("p (g f) -> p g f", g=GB),
                    u3_slice(k).rearrange("p (g f) -> p g f", g=GB),
                    _rep_outer(vb_sb[:], GB),
                    mybir.AluOpType.mult,
                )
            if not is_b:
                lg = b1_sb[:, k * GB * 32:(k + 1) * GB * 32]
                with nc.allow_low_precision(reason="bf16 logits store"):
                    nc.vector.tensor_reduce(
                        lg, t1[:].rearrange("p (x j) -> p x j", j=32),
                        mybir.AxisListType.X, mybir.AluOpType.add,
                    )
            else:
                b2 = smpool.tile([128, GB * 32], F32, tag="b2")
                nc.vector.tensor_reduce(
                    b2[:], t1[:].rearrange("p (x j) -> p x j", j=32),
                    mybir.AxisListType.X, mybir.AluOpType.add,
                )
                lg = b2[:]
                nc.vector.tensor_add(
                    lg, lg, b1_sb[:, k * GB * 32:(k + 1) * GB * 32]
                )
            e3 = epool.tile([128, GB * 32], BF16, tag="e3")
            nc.scalar.activation(
                e3[:], lg, mybir.ActivationFunctionType.Exp,
            )
            z3 = smpool.tile([128, GB], F32, tag="z3")
            nc.vector.tensor_reduce(
                z3[:], e3[:].rearrange("p (g c) -> p g c", g=GB),
                mybir.AxisListType.X, mybir.AluOpType.add,
            )
            r3 = smpool.tile([128, GB], F32, tag="r3")
            nc.vector.reciprocal(r3[:], z3[:])
            c3 = epool.tile([128, GB * 32], BF16, tag="c3")
            nc.vector.tensor_tensor(
                c3[:].rearrange("p (g c) -> p g c", g=GB),
                e3[:].rearrange("p (g c) -> p g c", g=GB),
                _bcast_inner(r3[:], 32),
                mybir.AluOpType.mult,
            )
            return c3

        def bstage2(k, ps_s, c3):
            t2 = btpool.tile([128, GB * 1024], BF16, tag="bt2")
            eng = (
                nc.gpsimd if (k * t2_pool_num) % t2_pool_den < t2_pool_num
                else nc.vector
            )
            if batch_mx:
                mx = btpool.tile([128, GB * 1024], BF16, tag="bmx")
                nc.scalar.copy(
                    mx[:].rearrange("p (g c j) -> p g c j", g=GB, c=32),
                    _bcast_inner(c3[:].rearrange("p (g c) -> p g c", g=GB), 32),
                )
                eng.tensor_tensor(
                    t2[:], u3_slice(k), mx[:], mybir.AluOpType.mult
                )
            else:
                eng.tensor_tensor(
                    t2[:].rearrange("p (x j) -> p x j", j=32),
                    u3_slice(k).rearrange("p (x j) -> p x j", j=32),
                    _bcast_inner(c3[:], 32),
                    mybir.AluOpType.mult,
                )
            for j in range(GB):
                for h in range(2):
                    sl = slice(j * 1024 + h * 512, j * 1024 + (h + 1) * 512)
                    nc.tensor.matmul(
                        ps_s[:, h * 512:(h + 1) * 512], delta_sb[:], t2[:, sl],
                        start=(k == 0 and j == 0),
                        stop=(k == NB - 1 and j == GB - 1),
                    )

        def sweep_batch(is_b):
            ps_s = psum_s.tile([BLOC, 1024], F32, tag="s")
            prev = None
            for k in range(NB + 1):
                if k < NB:
                    c3 = bstage1(k, is_b)
                if prev is not None:
                    bstage2(k - 1, ps_s, prev)
                prev = c3 if k < NB else None
            return ps_s

        def sweep(is_b):
            if sweep_mode == "batch":
                return sweep_batch(is_b)
            ps_s = psum_s.tile([BLOC, 1024], F32, tag="s")
            if sweep_mode == "chain":
                for g in range(NG):
                    mult_g, stat_g = stage1(g, is_b)
                    stage2(g, ps_s, t2_engine(g), mult_g, stat_g)
            else:
                handles = []
                for g in range(NG):
                    handles.append(stage1(g, is_b))
                for g in range(NG):
                    stage2(g, ps_s, t2_engine(g), *handles[g])
            return ps_s

        # ---------- Routing ----------
        for _rep in range(repeat):
            if empty:
                nc.vector.tensor_scalar(
                    s_sb[:], s_sb[:], 1.0, None, mybir.AluOpType.mult
                )
                nc.sync.dma_start(out=out_ext[:], in_=s_sb[:])
                continue
            if do_build:
                ps = build_uhat()
            else:
                ps = psum_s.tile([BLOC, 1024], F32, tag="s")
                nc.vector.tensor_scalar(
                    ps[:], s_sb[:], 1.0, None, mybir.AluOpType.mult
                )
            if n_sweeps >= 1:
                allreduce_squash(ps, 1.0 / C)
                ps = sweep(is_b=False)
            if n_sweeps >= 2:
                allreduce_squash(ps, 1.0)
                ps = sweep(is_b=not dup_a)
            nc.vector.tensor_copy(s_sb[:], ps[:])
            nc.sync.dma_start(out=out_ext[:], in_=s_sb[:])

    _split_multi_waits(nc)
    return nc


def host_prep(inputs, W, core):
    q4, g2 = core // 2, core % 2
    n0 = g2 * NLOC
    b0 = q4 * BLOC
    Wk = W[:, n0:n0 + NLOC]                                # [C, 576, Dc, Di]
    xk = inputs[b0:b0 + BLOC, n0:n0 + NLOC]                # [16, 576, Di]

    # w_pack[g, n8*16+i, c*32+j] = W[c, n0+g*8+n8, j, i]   (c-major free)
    wg = Wk.reshape(C, NG, 8, Dc, Di)                      # c g n8 j i
    w_pack = np.ascontiguousarray(
        wg.transpose(1, 2, 4, 0, 3).reshape(NG, 128, 1024)
    ).astype(ml_dtypes.bfloat16)

    # x arranged [g, n8, i, b]
    xg = xk.reshape(BLOC, NG, 8, Di).transpose(1, 2, 3, 0)  # g n8 i b
    x_pack = np.zeros((NG, 128, 144), dtype=np.float32)
    for n8 in range(8):
        x_pack[:, n8 * 16:(n8 + 1) * 16, n8 * 16:(n8 + 1) * 16] = xg[:, n8]
    x_pack[:, :, 128:144] = xg.reshape(NG, 128, 16)
    x_pack = x_pack.astype(ml_dtypes.bfloat16)

    delta_np = np.zeros((128, 16), dtype=np.float32)
    for n8 in range(8):
        for b16 in range(16):
            delta_np[n8 * 16 + b16, b16] = 1.0
    delta_np = delta_np.astype(ml_dtypes.bfloat16)

    return {"w_pack": w_pack, "x_pack": x_pack, "delta": delta_np}


def postprocess(partials):
    """partials[core] = s2 partial [16, 1024] (c-major).  Sum n-half pairs,
    squash, assemble [B, C, Dc]."""
    out = np.zeros((B, C, Dc), dtype=np.float32)
    for q4 in range(K4):
        s = (np.asarray(partials[q4 * 2], np.float32)
             + np.asarray(partials[q4 * 2 + 1], np.float32))
        s = s.reshape(BLOC, C, Dc)
        n2 = np.sum(s * s, axis=-1, keepdims=True)
        out[q4 * BLOC:(q4 + 1) * BLOC] = s * (
            n2 / (1.0 + n2) / np.sqrt(n2 + EPS)
        )
    return out


_NC_CACHE = {}


def _get_nc():
    if "nc" not in _NC_CACHE:
        _NC_CACHE["nc"] = build_program()
    return _NC_CACHE["nc"]


def kernel(inputs, W, _trace=False):
    inputs = np.asarray(inputs, dtype=np.float32)
    W = np.asarray(W, dtype=np.float32)
    nc = _get_nc()
    in_maps = [host_prep(inputs, W, k) for k in range(NCORES)]
    res = run_bass_kernel_spmd(
        nc, in_maps, core_ids=list(range(NCORES)), trace=_trace
    )
    kernel.last_results = res
    return postprocess([res.results[k]["out"] for k in range(NCORES)])


if __name__ == "__main__":
    rng = np.random.default_rng(0)
    x = rng.normal(size=(B, N, Di)).astype(np.float32)
    w = (rng.normal(size=(C, N, Dc, Di)) / np.sqrt(Di)).astype(np.float32)
    out = kernel(x, w)
    print("out", out.shape, out.dtype, np.abs(out).max())
